# revision 26
# baseline (speedup 1.0000x reference)
"""Trainium2 Bass kernel for nn_AttentionModule (B=8, C=256, L=2048, D=32).

Per-batch computation (data-parallel: one batch per NeuronCore, 8 cores):
    qT = Wq @ x + bq            # (D, L)
    kT = Wk @ x + bk            # (D, L)
    vT = x.T @ Wv.T             # (L, C)   -- bias bv handled algebraically:
                                #   out = v @ attn^T = Wv x E^T/Z + bv (rows of
                                #   attn sum to 1), so bv folds into the
                                #   residual as xr = x + gamma*bv host-side.
    ST = kT.T @ qT              # (L_j, L_i) = S[i,j] transposed
    E  = exp(ST)                # no max-subtraction: max|S| ~ 46, exp fits fp32
    Z  = sum_j E[j, i]          # column-tiled PE ones-matmuls -> PSUM partials
    U  = vT.T @ E               # (C, L_i)
    y  = gamma * U / Z + xr

Block (qd, J) covers j-blocks {4J+g} (J-major), so the first score matmuls
need only k-projection chunk 0 instead of all four: the exp stream starts
right after the first 512 columns of x land, ~4us earlier than a g-major
mapping. kT4's strip layout absorbs this via a diagonal gather in the k-bias
copyback (strip g of col-block J = replica g, cols 128g of projection chunk
J).

Engine split: PE does all matmuls including the Z partial reduction (4
column-tiled concurrent ones-matmuls per block accumulating into one
PSUM bank), the final Z reduce + transpose (so the reciprocal runs
128-wide) and the 1/Z broadcast (one K=1 N=512 matmul), all with bf16
stationary data (1 cyc/row). ACT does exp (the pacing producer: ~1.1us per
[128,1024] tile), half the k-bias copybacks, 4 vT casts, and the last
quarter's tail copies. DVE does the q-bias copyback, 12 vT casts, Z/U/rd
copybacks, reciprocal, and the last two quarters' y finalize; GPSIMD
finalizes quarters 0-1 (hidden under later compute).

Schedule: 16 (quarter, J) blocks are software-pipelined - scores+exp
for block k+1 are emitted before block k's U/Z matmuls, and each
quarter's tail matmuls are deferred into the next quarter's first block
so they can never head-of-line block the score pipeline on the PE
queue. Block 15 instead splits its Z matmuls between the two U half-streams
and inlines the tail, so the final Z->1/Z->broadcast->y chain starts ~1us
earlier and nothing idles behind it. Wq/Wk are replicated 4x host-side so
the projections emit qT4 (replicated strips) and kT4 directly. xb streams
in column chunks; ~20 dummy matmuls on a memset scratch tile (no DMA
dependency) warm the PE's HAM clock gate to 2.4GHz before the projections.
bv folds into the residual (attention rows sum to 1) and gamma into the
transpose identity, so no separate bias/scale passes exist anywhere.
"""

import numpy as np

B, C, L, D = 8, 256, 2048, 32
NCORES = 8

_cache = {}


def _build_nc():
    from contextlib import ExitStack

    import concourse.bacc as bacc
    import concourse.tile as tile
    from concourse import mybir

    f32 = mybir.dt.float32
    bf16 = mybir.dt.bfloat16
    EXP = mybir.ActivationFunctionType.Exp
    IDENT = mybir.ActivationFunctionType.Identity

    nc = bacc.Bacc("TRN2", target_bir_lowering=False, debug=False)

    xr_d = nc.dram_tensor("xr", [C, L], f32, kind="ExternalInput")
    xb_d = nc.dram_tensor("xb", [C, L], bf16, kind="ExternalInput")
    wpack_d = nc.dram_tensor("wpack", [C, 512], bf16, kind="ExternalInput")
    bqk_d = nc.dram_tensor("bqk", [128, 2], f32, kind="ExternalInput")
    # consts packed: col 0 = indicator (1 at rows 0,32,64,96), cols 1:129 =
    # gamma*I, cols 129:257 = ones (Z matmul lhsT uses 32 cols;
    # row 0 doubles as the broadcast lhsT onesr).
    cpack_d = nc.dram_tensor("cpack", [128, 257], bf16, kind="ExternalInput")
    y_d = nc.dram_tensor("y", [C, L], f32, kind="ExternalOutput")

    xr_ap = xr_d.ap()
    y_ap = y_d.ap()

    with tile.TileContext(nc) as tc, ExitStack() as ctx:
        singles = ctx.enter_context(tc.tile_pool(name="singles", bufs=1))
        big = ctx.enter_context(tc.tile_pool(name="big", bufs=1))
        ps = ctx.enter_context(tc.tile_pool(name="ps", bufs=2, space="PSUM"))
        up = ctx.enter_context(tc.tile_pool(name="up", bufs=1, space="PSUM"))
        zp = ctx.enter_context(tc.tile_pool(name="zp", bufs=2, space="PSUM"))
        epool = ctx.enter_context(tc.tile_pool(name="epool", bufs=10))
        ypool = ctx.enter_context(tc.tile_pool(name="ypool", bufs=4))
        uspool = ctx.enter_context(tc.tile_pool(name="uspool", bufs=2))
        rpool = ctx.enter_context(tc.tile_pool(name="rpool", bufs=2))

        # scratch for the HAM warmup: memset (no DMA dependency) so the
        # dummy matmuls start right after the framework preamble.
        scr_sb = singles.tile([128, 256], bf16, tag="scr")
        nc.gpsimd.memset(scr_sb[:], 0.0)

        # ---- loads: critical tensors first; the two HW DMA queues share the
        # HBM domain with the 7 sibling cores (~116GB/s effective), so what
        # matters is byte order, not queue count. First projection chunk
        # needs wpack (q/k cols) + xb cols 0:1024 of both channel chunks.
        wpack_sb, xb_sb = [], []
        for ct in range(2):
            tw = singles.tile([128, 512], bf16, tag=f"wp{ct}", name=f"wp{ct}")
            nc.sync.dma_start(out=tw[:], in_=wpack_d.ap()[ct * 128:(ct + 1) * 128, :])
            wpack_sb.append(tw)
        cpack_sb = singles.tile([128, 257], bf16, tag="cpack")
        nc.scalar.dma_start(out=cpack_sb[:], in_=cpack_d.ap()[:, :])
        bqk_sb = singles.tile([128, 2], f32, tag="bqk")
        nc.scalar.dma_start(out=bqk_sb[:], in_=bqk_d.ap()[:, :])
        for ct in range(2):
            xb_sb.append(big.tile([128, L], bf16, tag=f"xb{ct}", name=f"xb{ct}"))
        for half in range(2):
            for ct in range(2):
                nc.sync.dma_start(
                    out=xb_sb[ct][:, half * 1024:(half + 1) * 1024],
                    in_=xb_d.ap()[ct * 128:(ct + 1) * 128, half * 1024:(half + 1) * 1024],
                )
        indq_sb = cpack_sb[:, 0:1]
        identg_sb = cpack_sb[:, 1:129]
        ones_sb = cpack_sb[:, 129:161]
        onesr_sb = cpack_sb[0:1, 129:257]

        # ~4us of dummy matmuls on the scratch tile while the DMAs stream in:
        # pushes the PE's HAM clock-gate to full speed so the projections
        # run at 2.4GHz instead of 1.2.
        warm = ps.tile([128, 256], f32, tag="ps", name="warm")
        for w in range(16):
            nc.tensor.matmul(
                warm[:, :],
                lhsT=scr_sb[:, 0:128],
                rhs=scr_sb[:, 0:256],
                start=True,
                stop=True,
            )

        # ---- projections (PSUM borrowed from the U-accumulator pool) ----
        # Wq/Wk are replicated 4x host-side (wqk = [C, 128+128]) so the
        # projection matmuls directly produce qT replicated across the four
        # 32-partition strips (qT4) and kT in per-strip j-block layout (kT4):
        # no SBUF->SBUF replicate/rearrange DMAs at all.
        qT4 = big.tile([128, L], bf16, tag="qT4")
        # kT4: strip g at col-block J holds j-block jb = 4J+g (J-major), i.e.
        # rows 128g:128g+128 of projection chunk J; written by a 4-way
        # diagonal gather from the replicated k-projection PSUM.
        kT4 = big.tile([128, 512], bf16, tag="kT4")

        # the q/k projection chunks spread over all four PSUM chains (up x2,
        # zp x2) as their FIRST links, so the four chunk pipelines run in
        # parallel instead of ping-ponging through two buffers.
        def proj_psum(pool, name):
            if pool == "u0" or pool == "u1":
                return up.tile([128, 512], f32, tag=pool, name=name, bufs=1)
            if pool == "z":
                return zp.tile([128, 512], f32, tag="z", name=name)
            return ps.tile([128, 512], f32, tag="ps", name=name)

        def emit_s_exp(k):
            qd, J = divmod(k, 4)
            i0 = qd * 512
            e_tiles = []
            for pair in range(2):
                stp = ps.tile([128, 1024], f32, tag="ps")
                for h in range(2):
                    g = 2 * pair + h
                    nc.tensor.matmul(
                        stp[:, h * 512:(h + 1) * 512],
                        lhsT=kT4[32 * g:32 * (g + 1), J * 128:(J + 1) * 128],
                        rhs=qT4[32 * g:32 * (g + 1), i0:i0 + 512],
                        start=True,
                        stop=True,
                        tile_position=(32 * g, 0),
                    )
                e2 = epool.tile([128, 1024], bf16, tag="e")
                nc.scalar.activation(e2[:], stp[:], EXP)
                e_tiles.append(e2)
            return e_tiles

        # scores + exp for block k are emitted right after projection chunk k
        # (J-major block map: block (0, J) needs only kT4 col-block J = chunk
        # J and qT4 quarter 0 = chunk 0), so the exp stream starts as soon as
        # the first 512 columns of x are projected instead of after all four
        # chunks + the v-projection.
        hoisted_e = []
        PQ_POOL = {0: "u0", 1: "ps", 2: "z", 3: "z"}
        PK_POOL = {0: "u1", 1: "z", 2: "z", 3: "ps"}
        # The k bias bk is dropped entirely: S[j,i] = q_i . (k_j + bk) adds
        # q_i . bk, constant over j, which the softmax cancels (it scales E
        # and Z by the same e^{q.bk} ~ e^{+-0.3} per column).
        # k-projection: 4 column-tiled CONCURRENT matmuls per chunk, replica
        # strip g streaming its own 128-col slice of x, so the PSUM lands
        # directly in kT4's diagonal (strip g = j-block 4*it+g) layout AND
        # the k-proj streams 256 cols/chunk instead of 1024.
        for it in range(4):
            pq = proj_psum(PQ_POOL[it], f"pq{it}")
            pk = proj_psum(PK_POOL[it], f"pk{it}")
            for ct in range(2):
                nc.tensor.matmul(
                    pq[:, :],
                    lhsT=wpack_sb[ct][:, 0:128],
                    rhs=xb_sb[ct][:, it * 512:(it + 1) * 512],
                    start=(ct == 0),
                    stop=(ct == 1),
                )
            for ct in range(2):
                for g in range(4):
                    nc.tensor.matmul(
                        pk[32 * g:32 * (g + 1), 0:128],
                        lhsT=wpack_sb[ct][:, 128 + 32 * g:128 + 32 * (g + 1)],
                        rhs=xb_sb[ct][:, it * 512 + 128 * g:it * 512 + 128 * (g + 1)],
                        start=(ct == 0),
                        stop=(ct == 1),
                        tile_position=(0, 32 * g),
                        skip_group_check=True,
                    )
            nc.vector.tensor_scalar_add(
                qT4[:, it * 512:(it + 1) * 512], pq[:, :], bqk_sb[:, 0:1]
            )
            # kT4 cast on ACT for even chunks so chunk 0's cast runs in
            # parallel with its q-bias on DVE (S(0) gates on both)
            if it % 2 == 0:
                nc.scalar.copy(kT4[:, 128 * it:128 * (it + 1)], pk[:, 0:128])
            else:
                nc.vector.tensor_copy(kT4[:, 128 * it:128 * (it + 1)], pk[:, 0:128])
            hoisted_e.append(emit_s_exp(it))

        # vT[j, c] stored as [128, 16*256]: block jb holds vT[jb*128 + p, c].
        # PSUM rotates over four independent chains (u0, u1, and the two idle
        # zp slots) so the per-chain MM->cast serial chains run in parallel;
        # block (qd, J) consumes lb in {4J..4J+3}, one from each chain.
        # v-projection in first-needed order under the J-major block map
        # (block 0 consumes lb 0..3), chains balanced like the baseline
        # g-major map via the same transposed assignment.
        vT_sb = big.tile([128, 16 * C], bf16, tag="vT")
        VP_ORDER = [2, 3, 0, 1, 4, 5, 8, 12, 9, 13, 7, 6, 11, 10, 14, 15]
        VP_POOL = {0: "u0", 1: "u1", 2: "z", 3: "z", 8: "z", 12: "z",
                   9: "z", 13: "z", 10: "z", 14: "z",
                   4: "ps", 5: "ps", 7: "ps", 6: "ps", 11: "ps", 15: "ps"}
        for nv, lb in enumerate(VP_ORDER):
            p = proj_psum(VP_POOL[lb], f"pv{lb}")
            for ct in range(2):
                nc.tensor.matmul(
                    p[:, :C],
                    lhsT=xb_sb[ct][:, lb * 128:(lb + 1) * 128],
                    rhs=wpack_sb[ct][:, 256:512],
                    start=(ct == 0),
                    stop=(ct == 1),
                )
            # the casts gating the first score tiles go to ACT
            if lb in (14, 15):
                nc.scalar.copy(vT_sb[:, lb * C:(lb + 1) * C], p[:, :C])
            else:
                nc.vector.tensor_copy(vT_sb[:, lb * C:(lb + 1) * C], p[:, :C])
            # feed the exp stream through the v-projection: scores for blocks
            # 4 and 5 interleave into the PE queue here so ACT never runs dry
            # while the v-projection + first quarter's U matmuls drain.
            if nv == 7:
                hoisted_e.append(emit_s_exp(4))
            elif nv == 11:
                hoisted_e.append(emit_s_exp(5))

        # fp32 residual xr = x + gamma*bv (prepared host-side)
        xr_sb = []
        for ct in range(2):
            t = big.tile([128, L], f32, tag=f"x{ct}")
            nc.sync.dma_start(out=t[:], in_=xr_ap[ct * 128:(ct + 1) * 128, :])
            xr_sb.append(t)

        # ---- attention, processed in i-quarters of 512 columns ----
        # emit_tail: reduce the 4 Z partial rows, 1/Z, transpose+broadcast via
        # PE, then y = us*(gamma/Z) + xr. Quarters 0-1 finalize on GPSIMD
        # (hidden under later compute); quarters 2-3 on DVE straight from PSUM.
        def emit_tail_a(u_t, zpart, last=False):
            # phase A (at J==3): free the U banks, copy Z partials to SBUF,
            # and allocate the tail PSUM tiles (keeping the zp pool rotation:
            # zpart, zt, rd_ps, rb_ps per quarter).
            zsb = rpool.tile([128, 512], bf16, tag="zsb", name="zsb")
            if last:
                # the final chain is pure latency: zsb halves go FIRST on
                # both ACT+DVE (the zt matmuls gate on it), us copies after
                # (their consumer, the y multiply, comes ~2us later)
                nc.scalar.copy(zsb[0:64, :], zpart[0:64, :])
                nc.vector.tensor_copy(zsb[64:128, :], zpart[64:128, :])
            us = []
            for ct in range(2):
                u = uspool.tile([128, 512], f32, tag=f"us{ct}", name=f"us{ct}")
                if last and ct == 0:
                    nc.scalar.copy(u[:], u_t[ct][:, :])
                else:
                    nc.vector.tensor_copy(u[:], u_t[ct][:, :])
                us.append(u)
            if not last:
                nc.vector.tensor_copy(zsb[:], zpart[:, :])
            # allocation order is load-bearing: with [rd_ps, zt, rb_ps] the
            # NEXT quarter's zpart inherits zt's bank (freed by the reciprocal
            # ~1.1us into this tail) instead of rd_ps's (freed by the rd cast
            # ~2.3us in), so the new quarter's first Z matmuls never stall on
            # this quarter's tail chain.
            # the last quarter uses column-tiled concurrent rd matmuls whose
            # chunks land at psum partitions 32c, so its rd_ps spans all
            # partitions; mid-kernel quarters keep the [1,512] shape that
            # preserves the zp bank rotation.
            rd_ps = zp.tile([128 if last else 1, 512], f32, tag="z", name="rd_ps")
            zt = zp.tile([128, 4], f32, tag="z", name="zt")
            rb_ps = zp.tile([128, 512], f32, tag="z", name="rb_ps")
            return (us, zsb, zt, rd_ps, rb_ps)

        def emit_tail_b(i0, pend, on_dve, last=False):
            # phase B (emitted after the NEXT block's U stream so none of
            # these PE matmuls can head-of-line block the score pipeline):
            # Z reduce+transpose, 1/Z, broadcast, y = us*(gamma/Z) + xr.
            us, zsb, zt, rd_ps, rb_ps = pend
            for c in range(4):
                nc.tensor.matmul(
                    zt[:, c:c + 1],
                    lhsT=zsb[:, 128 * c:128 * (c + 1)],
                    rhs=indq_sb,
                    start=True,
                    stop=True,
                )
            rt = rpool.tile([128, 4], bf16, tag="rt", name="rt")
            with nc.allow_low_precision(reason="1/Z in bf16: 0.4% on a 2e-2 budget"):
                nc.vector.reciprocal(rt[:], zt[:, 0:4])
            rd = rpool.tile([1, 512], bf16, tag="rd", name="rd")
            if last:
                # 4 column-tiled CONCURRENT rd matmuls (chunk c at psum
                # partition 32c) + 4 small casts split DVE/ACT: ~1us off the
                # pure-latency final chain.
                for c in range(4):
                    nc.tensor.matmul(
                        rd_ps[32 * c:32 * c + 1, 128 * c:128 * (c + 1)],
                        lhsT=rt[:, c:c + 1],
                        rhs=identg_sb,
                        start=True,
                        stop=True,
                        tile_position=(0, 32 * c),
                    )
                for c in range(4):
                    src = rd_ps[32 * c:32 * c + 1, 128 * c:128 * (c + 1)]
                    dst = rd[0:1, 128 * c:128 * (c + 1)]
                    if c % 2 == 0:
                        nc.vector.tensor_copy(dst, src)
                    else:
                        nc.scalar.copy(dst, src)
            else:
                for c in range(4):
                    nc.tensor.matmul(
                        rd_ps[0:1, 128 * c:128 * (c + 1)],
                        lhsT=rt[:, c:c + 1],
                        rhs=identg_sb,
                        start=True,
                        stop=True,
                    )
                nc.vector.tensor_copy(rd[:], rd_ps[0:1, :])
            # single K=1 N=512 broadcast matmul (the 4-chunk split buys
            # nothing once rd exists as one row)
            nc.tensor.matmul(
                rb_ps[:, :],
                lhsT=onesr_sb,
                rhs=rd[0:1, :],
                start=True,
                stop=True,
            )
            if not on_dve:
                rb_sb = rpool.tile([128, 512], f32, tag="rb", name="rb_sb")
                nc.vector.tensor_copy(rb_sb[:], rb_ps[:, :])
            eng = nc.vector if on_dve else nc.gpsimd
            for ct in range(2):
                yt = ypool.tile([128, 512], f32, tag="y", name="yt")
                eng.tensor_mul(yt[:], us[ct][:], rb_ps[:, :] if on_dve else rb_sb[:])
                eng.tensor_add(yt[:], yt[:], xr_sb[ct][:, i0:i0 + 512])
                nc.sync.dma_start(
                    out=y_ap[ct * 128:(ct + 1) * 128, i0:i0 + 512], in_=yt[:]
                )

        # Software pipeline over 16 (quarter, J) blocks: the scores + exp for
        # block k+1 are emitted at high priority BEFORE block k's U/Z matmuls,
        # so the next block's scores can never be head-of-line blocked behind
        # this block's U stream or the quarter tail on the PE queue.
        u_t = None
        zpart = None
        pending = None
        e_cur = hoisted_e[0]
        for k in range(16):
            qd, J = divmod(k, 4)
            i0 = qd * 512
            if J == 0:
                u_t = [
                    up.tile([128, 512], f32, tag=f"u{ct}", name=f"u{ct}", bufs=1)
                    for ct in range(2)
                ]
                # Z partials: row band 32g accumulates sum over J of E[4J+g];
                # written by column-tiled concurrent ones-matmuls.
                zpart = zp.tile([128, 512], f32, tag="z", name="zpart")
            if k + 1 < len(hoisted_e):
                e_next = hoisted_e[k + 1]
            else:
                e_next = emit_s_exp(k + 1) if k < 15 else None

            def emit_u(gs):
                for g in gs:
                    jb = 4 * J + g
                    eh = e_cur[g // 2][:, (g % 2) * 512:(g % 2 + 1) * 512]
                    for ct in range(2):
                        nc.tensor.matmul(
                            u_t[ct][:, :],
                            lhsT=vT_sb[:, jb * C + ct * 128:jb * C + ct * 128 + 128],
                            rhs=eh,
                            start=(jb == 0),
                            stop=(jb == 15),
                        )

            def emit_z(gs):
                for g in gs:
                    eh = e_cur[g // 2][:, (g % 2) * 512:(g % 2 + 1) * 512]
                    nc.tensor.matmul(
                        zpart[32 * g:32 * (g + 1), :],
                        lhsT=ones_sb,
                        rhs=eh,
                        start=(J == 0),
                        stop=(J == 3),
                        tile_position=(0, 32 * g),
                        # 4 col-groups accumulate into disjoint 32-row bands of
                        # one bank; per-element has_written makes this safe on
                        # HW but the sim's group map is partition-blind.
                        skip_group_check=True,
                    )

            if k < 15:
                emit_u(range(4))
                # the 4 Z matmuls back-to-back so they pack into one
                # concurrent column-tiled group (~1 matmul of PE time for 4)
                emit_z(range(4))
            else:
                # last block: Z ahead of the second U half-stream (same exp
                # gate) so zpart completes ~0.9us before the U stream ends
                # and the final tail chain overlaps the last U matmuls.
                emit_u((0, 1))
                emit_z((0, 1))
                emit_z((2, 3))
                emit_u((2, 3))
            e_cur = e_next
            if pending is not None:
                # previous quarter's tail (pending is set at J==3, consumed at
                # the next block = J==0 of quarter qd), now safely behind this
                # block's score+U matmuls in queue order
                emit_tail_b((qd - 1) * 512, pending, on_dve=(qd - 1 >= 2))
                pending = None
            if J == 3:
                pending = emit_tail_a(u_t, zpart, last=(k == 15))
                if k == 15:
                    emit_tail_b(i0, pending, on_dve=True, last=True)
                    pending = None

    nc.compile()
    return nc


def get_nc():
    if "nc" not in _cache:
        _cache["nc"] = _build_nc()
    return _cache["nc"]


def make_in_maps(x, Wq, bq, Wk, bk, Wv, bv, gamma):
    import ml_dtypes

    bf = ml_dtypes.bfloat16
    x = np.asarray(x, dtype=np.float32)
    g = float(np.asarray(gamma, np.float32).reshape(-1)[0])
    cpack = np.zeros((128, 257), np.float32)
    cpack[0::32, 0] = 1.0                       # indicator
    cpack[:, 1:129] = g * np.eye(128)           # gamma * I
    cpack[:, 129:257] = 1.0                     # ones
    # bv folds into the residual: out = gamma*(Wv x E^T)/Z + (x + gamma*bv)
    xr_bias = (g * np.asarray(bv, np.float32)).reshape(1, C, 1)
    shared = {
        "wpack": np.ascontiguousarray(
            np.concatenate([np.tile(np.asarray(Wq, np.float32).T, (1, 4)),
                            np.tile(np.asarray(Wk, np.float32).T, (1, 4)),
                            np.asarray(Wv, np.float32).T],
                           axis=1)).astype(bf),
        "bqk": np.stack([np.tile(np.asarray(bq, np.float32), 4),
                         np.tile(np.asarray(bk, np.float32), 4)], axis=1),
        "cpack": cpack.astype(bf),
    }
    xrs = (x + xr_bias).astype(np.float32)
    return [
        dict(shared, xr=np.ascontiguousarray(xrs[b]),
             xb=np.ascontiguousarray(x[b]).astype(bf))
        for b in range(B)
    ]


def kernel(x, Wq, bq, Wk, bk, Wv, bv, gamma):
    from concourse.bass_utils import run_bass_kernel_spmd

    nc = get_nc()
    in_maps = make_in_maps(x, Wq, bq, Wk, bk, Wv, bv, gamma)
    res = run_bass_kernel_spmd(nc, in_maps, list(range(NCORES)))
    return np.stack([res.results[b]["y"] for b in range(B)], axis=0)


# revision 28
# speedup vs baseline: 1.1812x; 1.1812x over previous
"""Trainium2 Bass kernel for nn_AttentionModule (B=8, C=256, L=2048, D=32).

Per-batch computation (data-parallel: one batch per NeuronCore, 8 cores):
    qT = Wq @ x + bq            # (D, L)
    kT = Wk @ x + bk            # (D, L)
    vT = x.T @ Wv.T             # (L, C)   -- bias bv handled algebraically:
                                #   out = v @ attn^T = Wv x E^T/Z + bv (rows of
                                #   attn sum to 1), so bv folds into the
                                #   residual as xr = x + gamma*bv host-side.
    ST = kT.T @ qT              # (L_j, L_i) = S[i,j] transposed
    E  = exp(ST)                # no max-subtraction: max|S| ~ 46, exp fits fp32
    Z  = sum_j E[j, i]          # column-tiled PE ones-matmuls -> PSUM partials
    U  = vT.T @ E               # (C, L_i)
    y  = gamma * U / Z + xr

Block (qd, J) covers j-blocks {4J+g} (J-major), so the first score matmuls
need only k-projection chunk 0 instead of all four: the exp stream starts
right after the first 512 columns of x land, ~4us earlier than a g-major
mapping. kT4's strip layout absorbs this via a diagonal gather in the k-bias
copyback (strip g of col-block J = replica g, cols 128g of projection chunk
J).

Engine split: PE does all matmuls including the Z partial reduction (4
column-tiled concurrent ones-matmuls per block accumulating into one
PSUM bank), the final Z reduce + transpose (so the reciprocal runs
128-wide) and the 1/Z broadcast (one K=1 N=512 matmul), all with bf16
stationary data (1 cyc/row). ACT does exp (the pacing producer: ~1.1us per
[128,1024] tile), half the k-bias copybacks, 4 vT casts, and the last
quarter's tail copies. DVE does the q-bias copyback, 12 vT casts, Z/U/rd
copybacks, reciprocal, and the last two quarters' y finalize; GPSIMD
finalizes quarters 0-1 (hidden under later compute).

Schedule: 16 (quarter, J) blocks are software-pipelined - scores+exp
for block k+1 are emitted before block k's U/Z matmuls, and each
quarter's tail matmuls are deferred into the next quarter's first block
so they can never head-of-line block the score pipeline on the PE
queue. Block 15 instead splits its Z matmuls between the two U half-streams
and inlines the tail, so the final Z->1/Z->broadcast->y chain starts ~1us
earlier and nothing idles behind it. Wq/Wk are replicated 4x host-side so
the projections emit qT4 (replicated strips) and kT4 directly. xb streams
in column chunks; ~20 dummy matmuls on a memset scratch tile (no DMA
dependency) warm the PE's HAM clock gate to 2.4GHz before the projections.
bv folds into the residual (attention rows sum to 1) and gamma into the
transpose identity, so no separate bias/scale passes exist anywhere.
"""

import numpy as np

B, C, L, D = 8, 256, 2048, 32
NCORES = 8

_cache = {}


def _build_nc():
    from contextlib import ExitStack

    import concourse.bacc as bacc
    import concourse.tile as tile
    from concourse import mybir

    f32 = mybir.dt.float32
    bf16 = mybir.dt.bfloat16
    EXP = mybir.ActivationFunctionType.Exp
    IDENT = mybir.ActivationFunctionType.Identity

    nc = bacc.Bacc("TRN2", target_bir_lowering=False, debug=False)

    xr_d = nc.dram_tensor("xr", [C, L], f32, kind="ExternalInput")
    xb_d = nc.dram_tensor("xb", [C, L], bf16, kind="ExternalInput")
    wpack_d = nc.dram_tensor("wpack", [C, 512], bf16, kind="ExternalInput")
    bqk_d = nc.dram_tensor("bqk", [128, 2], f32, kind="ExternalInput")
    # consts packed: col 0 = indicator (1 at rows 0,32,64,96), cols 1:129 =
    # gamma*I, cols 129:257 = ones (Z matmul lhsT uses 32 cols;
    # row 0 doubles as the broadcast lhsT onesr).
    cpack_d = nc.dram_tensor("cpack", [128, 257], bf16, kind="ExternalInput")
    y_d = nc.dram_tensor("y", [C, L], f32, kind="ExternalOutput")

    xr_ap = xr_d.ap()
    y_ap = y_d.ap()

    with tile.TileContext(nc) as tc, ExitStack() as ctx:
        singles = ctx.enter_context(tc.tile_pool(name="singles", bufs=1))
        big = ctx.enter_context(tc.tile_pool(name="big", bufs=1))
        ps = ctx.enter_context(tc.tile_pool(name="ps", bufs=2, space="PSUM"))
        up = ctx.enter_context(tc.tile_pool(name="up", bufs=1, space="PSUM"))
        zp = ctx.enter_context(tc.tile_pool(name="zp", bufs=2, space="PSUM"))
        epool = ctx.enter_context(tc.tile_pool(name="epool", bufs=10))
        ypool = ctx.enter_context(tc.tile_pool(name="ypool", bufs=4))
        uspool = ctx.enter_context(tc.tile_pool(name="uspool", bufs=2))
        rpool = ctx.enter_context(tc.tile_pool(name="rpool", bufs=2))

        # scratch for the HAM warmup: memset (no DMA dependency) so the
        # dummy matmuls start right after the framework preamble.
        scr_sb = singles.tile([128, 256], bf16, tag="scr")
        nc.gpsimd.memset(scr_sb[:], 0.0)

        # ---- loads: critical tensors first; the two HW DMA queues share the
        # HBM domain with the 7 sibling cores (~116GB/s effective), so what
        # matters is byte order, not queue count. First projection chunk
        # needs wpack (q/k cols) + xb cols 0:1024 of both channel chunks.
        wpack_sb, xb_sb = [], []
        for ct in range(2):
            tw = singles.tile([128, 512], bf16, tag=f"wp{ct}", name=f"wp{ct}")
            nc.sync.dma_start(out=tw[:], in_=wpack_d.ap()[ct * 128:(ct + 1) * 128, :])
            wpack_sb.append(tw)
        cpack_sb = singles.tile([128, 257], bf16, tag="cpack")
        nc.scalar.dma_start(out=cpack_sb[:], in_=cpack_d.ap()[:, :])
        bqk_sb = singles.tile([128, 2], f32, tag="bqk")
        nc.scalar.dma_start(out=bqk_sb[:], in_=bqk_d.ap()[:, :])
        for ct in range(2):
            xb_sb.append(big.tile([128, L], bf16, tag=f"xb{ct}", name=f"xb{ct}"))
        for half in range(2):
            for ct in range(2):
                nc.sync.dma_start(
                    out=xb_sb[ct][:, half * 1024:(half + 1) * 1024],
                    in_=xb_d.ap()[ct * 128:(ct + 1) * 128, half * 1024:(half + 1) * 1024],
                )
        indq_sb = cpack_sb[:, 0:1]
        identg_sb = cpack_sb[:, 1:129]
        ones_sb = cpack_sb[:, 129:161]
        onesr_sb = cpack_sb[0:1, 129:257]

        # ~4us of dummy matmuls on the scratch tile while the DMAs stream in:
        # pushes the PE's HAM clock-gate to full speed so the projections
        # run at 2.4GHz instead of 1.2.
        warm = ps.tile([128, 256], f32, tag="ps", name="warm")
        for w in range(16):
            nc.tensor.matmul(
                warm[:, :],
                lhsT=scr_sb[:, 0:128],
                rhs=scr_sb[:, 0:256],
                start=True,
                stop=True,
            )

        # ---- projections (PSUM borrowed from the U-accumulator pool) ----
        # Wq/Wk are replicated 4x host-side (wqk = [C, 128+128]) so the
        # projection matmuls directly produce qT replicated across the four
        # 32-partition strips (qT4) and kT in per-strip j-block layout (kT4):
        # no SBUF->SBUF replicate/rearrange DMAs at all.
        qT4 = big.tile([128, L], bf16, tag="qT4")
        # kT4: strip g at col-block J holds j-block jb = 4J+g (J-major), i.e.
        # rows 128g:128g+128 of projection chunk J; written by a 4-way
        # diagonal gather from the replicated k-projection PSUM.
        kT4 = big.tile([128, 512], bf16, tag="kT4")

        # the q/k projection chunks spread over all four PSUM chains (up x2,
        # zp x2) as their FIRST links, so the four chunk pipelines run in
        # parallel instead of ping-ponging through two buffers.
        def proj_psum(pool, name):
            if pool == "u0" or pool == "u1":
                return up.tile([128, 512], f32, tag=pool, name=name, bufs=1)
            if pool == "z":
                return zp.tile([128, 512], f32, tag="z", name=name)
            return ps.tile([128, 512], f32, tag="ps", name=name)

        def emit_s_exp(k):
            qd, J = divmod(k, 4)
            i0 = qd * 512
            e_tiles = []
            for pair in range(2):
                stp = ps.tile([128, 1024], f32, tag="ps")
                for h in range(2):
                    g = 2 * pair + h
                    nc.tensor.matmul(
                        stp[:, h * 512:(h + 1) * 512],
                        lhsT=kT4[32 * g:32 * (g + 1), J * 128:(J + 1) * 128],
                        rhs=qT4[32 * g:32 * (g + 1), i0:i0 + 512],
                        start=True,
                        stop=True,
                        tile_position=(32 * g, 0),
                    )
                e2 = epool.tile([128, 1024], bf16, tag="e")
                nc.scalar.activation(e2[:], stp[:], EXP)
                e_tiles.append(e2)
            return e_tiles

        # vT[j, c] stored as [128, 16*256]: block jb holds vT[jb*128 + p, c].
        # v-projection chains use u0/u1/z PSUM only (never the stp-shared
        # 'ps' slots, except the last two blocks emitted after all stp
        # hoists) so interleaving v-projection with score hoists can't
        # couple a v-cast chain to the exp stream. Block (qd, J) consumes
        # lb in {4J..4J+3}; VP_A (blocks 0-7, needs only the first half of
        # x) fills the PE while the second half of x is still in flight.
        vT_sb = big.tile([128, 16 * C], bf16, tag="vT")
        VP_POOL = {0: "u0", 4: "u0", 8: "u0", 1: "u1", 5: "u1", 9: "u1",
                   2: "z", 3: "z", 6: "z", 7: "z", 10: "z", 11: "z",
                   12: "z", 13: "z", 14: "ps", 15: "ps"}
        VP_A = [2, 0, 1, 3, 4, 5, 6, 7]
        VP_B = [8, 9, 10, 11, 12, 13, 14, 15]

        def emit_vproj(lbs):
            for lb in lbs:
                p = proj_psum(VP_POOL[lb], f"pv{lb}")
                for ct in range(2):
                    nc.tensor.matmul(
                        p[:, :C],
                        lhsT=xb_sb[ct][:, lb * 128:(lb + 1) * 128],
                        rhs=wpack_sb[ct][:, 256:512],
                        start=(ct == 0),
                        stop=(ct == 1),
                    )
                nc.vector.tensor_copy(vT_sb[:, lb * C:(lb + 1) * C], p[:, :C])

        # scores + exp for block k are emitted right after projection chunk k
        # (J-major block map: block (0, J) needs only kT4 col-block J = chunk
        # J and qT4 quarter 0 = chunk 0), so the exp stream starts as soon as
        # the first 512 columns of x are projected instead of after all four
        # chunks + the v-projection.
        hoisted_e = []
        PQ_POOL = {0: "u0", 1: "ps", 2: "z", 3: "z"}
        PK_POOL = {0: "u1", 1: "z", 2: "z", 3: "ps"}
        # The k bias bk is dropped entirely: S[j,i] = q_i . (k_j + bk) adds
        # q_i . bk, constant over j, which the softmax cancels (it scales E
        # and Z by the same e^{q.bk} ~ e^{+-0.3} per column).
        # k-projection: 4 column-tiled CONCURRENT matmuls per chunk, replica
        # strip g streaming its own 128-col slice of x, so the PSUM lands
        # directly in kT4's diagonal (strip g = j-block 4*it+g) layout AND
        # the k-proj streams 256 cols/chunk instead of 1024.
        for it in range(4):
            pq = proj_psum(PQ_POOL[it], f"pq{it}")
            pk = proj_psum(PK_POOL[it], f"pk{it}")
            for ct in range(2):
                nc.tensor.matmul(
                    pq[:, :],
                    lhsT=wpack_sb[ct][:, 0:128],
                    rhs=xb_sb[ct][:, it * 512:(it + 1) * 512],
                    start=(ct == 0),
                    stop=(ct == 1),
                )
            for ct in range(2):
                for g in range(4):
                    nc.tensor.matmul(
                        pk[32 * g:32 * (g + 1), 0:128],
                        lhsT=wpack_sb[ct][:, 128 + 32 * g:128 + 32 * (g + 1)],
                        rhs=xb_sb[ct][:, it * 512 + 128 * g:it * 512 + 128 * (g + 1)],
                        start=(ct == 0),
                        stop=(ct == 1),
                        tile_position=(0, 32 * g),
                        skip_group_check=True,
                    )
            nc.vector.tensor_scalar_add(
                qT4[:, it * 512:(it + 1) * 512], pq[:, :], bqk_sb[:, 0:1]
            )
            # kT4 cast on ACT for even chunks so chunk 0's cast runs in
            # parallel with its q-bias on DVE (S(0) gates on both)
            if it % 2 == 0:
                nc.scalar.copy(kT4[:, 128 * it:128 * (it + 1)], pk[:, 0:128])
            else:
                nc.vector.tensor_copy(kT4[:, 128 * it:128 * (it + 1)], pk[:, 0:128])
            hoisted_e.append(emit_s_exp(it))
            if it == 1:
                emit_vproj(VP_A)

        emit_vproj(VP_B)

        # fp32 residual xr = x + gamma*bv (prepared host-side)
        xr_sb = []
        for ct in range(2):
            t = big.tile([128, L], f32, tag=f"x{ct}")
            nc.sync.dma_start(out=t[:], in_=xr_ap[ct * 128:(ct + 1) * 128, :])
            xr_sb.append(t)

        # ---- attention, processed in i-quarters of 512 columns ----
        # emit_tail: reduce the 4 Z partial rows, 1/Z, transpose+broadcast via
        # PE, then y = us*(gamma/Z) + xr. Quarters 0-1 finalize on GPSIMD
        # (hidden under later compute); quarters 2-3 on DVE straight from PSUM.
        def emit_tail_a(u_t, zpart, last=False):
            # phase A (at J==3): free the U banks, copy Z partials to SBUF,
            # and allocate the tail PSUM tiles (keeping the zp pool rotation:
            # zpart, zt, rd_ps, rb_ps per quarter).
            zsb = rpool.tile([128, 512], bf16, tag="zsb", name="zsb")
            if last:
                # the final chain is pure latency: zsb halves go FIRST on
                # both ACT+DVE (the zt matmuls gate on it), us copies after
                # (their consumer, the y multiply, comes ~2us later)
                nc.scalar.copy(zsb[0:64, :], zpart[0:64, :])
                nc.vector.tensor_copy(zsb[64:128, :], zpart[64:128, :])
            us = []
            for ct in range(2):
                u = uspool.tile([128, 512], f32, tag=f"us{ct}", name=f"us{ct}")
                if last and ct == 0:
                    nc.scalar.copy(u[:], u_t[ct][:, :])
                else:
                    nc.vector.tensor_copy(u[:], u_t[ct][:, :])
                us.append(u)
            if not last:
                nc.vector.tensor_copy(zsb[:], zpart[:, :])
            # allocation order is load-bearing: with [rd_ps, zt, rb_ps] the
            # NEXT quarter's zpart inherits zt's bank (freed by the reciprocal
            # ~1.1us into this tail) instead of rd_ps's (freed by the rd cast
            # ~2.3us in), so the new quarter's first Z matmuls never stall on
            # this quarter's tail chain.
            # the last quarter uses column-tiled concurrent rd matmuls whose
            # chunks land at psum partitions 32c, so its rd_ps spans all
            # partitions; mid-kernel quarters keep the [1,512] shape that
            # preserves the zp bank rotation.
            rd_ps = zp.tile([128 if last else 1, 512], f32, tag="z", name="rd_ps")
            zt = zp.tile([128, 4], f32, tag="z", name="zt")
            rb_ps = zp.tile([128, 512], f32, tag="z", name="rb_ps")
            return (us, zsb, zt, rd_ps, rb_ps)

        def emit_tail_b(i0, pend, on_dve, last=False):
            # phase B (emitted after the NEXT block's U stream so none of
            # these PE matmuls can head-of-line block the score pipeline):
            # Z reduce+transpose, 1/Z, broadcast, y = us*(gamma/Z) + xr.
            us, zsb, zt, rd_ps, rb_ps = pend
            for c in range(4):
                nc.tensor.matmul(
                    zt[:, c:c + 1],
                    lhsT=zsb[:, 128 * c:128 * (c + 1)],
                    rhs=indq_sb,
                    start=True,
                    stop=True,
                )
            rt = rpool.tile([128, 4], bf16, tag="rt", name="rt")
            with nc.allow_low_precision(reason="1/Z in bf16: 0.4% on a 2e-2 budget"):
                nc.vector.reciprocal(rt[:], zt[:, 0:4])
            rd = rpool.tile([1, 512], bf16, tag="rd", name="rd")
            if last:
                # 4 column-tiled CONCURRENT rd matmuls (chunk c at psum
                # partition 32c) + 4 small casts split DVE/ACT: ~1us off the
                # pure-latency final chain.
                for c in range(4):
                    nc.tensor.matmul(
                        rd_ps[32 * c:32 * c + 1, 128 * c:128 * (c + 1)],
                        lhsT=rt[:, c:c + 1],
                        rhs=identg_sb,
                        start=True,
                        stop=True,
                        tile_position=(0, 32 * c),
                    )
                for c in range(4):
                    src = rd_ps[32 * c:32 * c + 1, 128 * c:128 * (c + 1)]
                    dst = rd[0:1, 128 * c:128 * (c + 1)]
                    if c % 2 == 0:
                        nc.vector.tensor_copy(dst, src)
                    else:
                        nc.scalar.copy(dst, src)
            else:
                for c in range(4):
                    nc.tensor.matmul(
                        rd_ps[0:1, 128 * c:128 * (c + 1)],
                        lhsT=rt[:, c:c + 1],
                        rhs=identg_sb,
                        start=True,
                        stop=True,
                    )
                nc.vector.tensor_copy(rd[:], rd_ps[0:1, :])
            # single K=1 N=512 broadcast matmul (the 4-chunk split buys
            # nothing once rd exists as one row)
            nc.tensor.matmul(
                rb_ps[:, :],
                lhsT=onesr_sb,
                rhs=rd[0:1, :],
                start=True,
                stop=True,
            )
            if not on_dve:
                rb_sb = rpool.tile([128, 512], f32, tag="rb", name="rb_sb")
                nc.vector.tensor_copy(rb_sb[:], rb_ps[:, :])
            eng = nc.vector if on_dve else nc.gpsimd
            for ct in range(2):
                yt = ypool.tile([128, 512], f32, tag="y", name="yt")
                eng.tensor_mul(yt[:], us[ct][:], rb_ps[:, :] if on_dve else rb_sb[:])
                eng.tensor_add(yt[:], yt[:], xr_sb[ct][:, i0:i0 + 512])
                nc.sync.dma_start(
                    out=y_ap[ct * 128:(ct + 1) * 128, i0:i0 + 512], in_=yt[:]
                )

        # Software pipeline over 16 (quarter, J) blocks: the scores + exp for
        # block k+1 are emitted at high priority BEFORE block k's U/Z matmuls,
        # so the next block's scores can never be head-of-line blocked behind
        # this block's U stream or the quarter tail on the PE queue.
        u_t = None
        zpart = None
        pending = None
        e_cur = hoisted_e[0]
        for k in range(16):
            qd, J = divmod(k, 4)
            i0 = qd * 512
            if J == 0:
                u_t = [
                    up.tile([128, 512], f32, tag=f"u{ct}", name=f"u{ct}", bufs=1)
                    for ct in range(2)
                ]
                # Z partials: row band 32g accumulates sum over J of E[4J+g];
                # written by column-tiled concurrent ones-matmuls.
                zpart = zp.tile([128, 512], f32, tag="z", name="zpart")
            if k + 1 < len(hoisted_e):
                e_next = hoisted_e[k + 1]
            else:
                e_next = emit_s_exp(k + 1) if k < 15 else None

            def emit_u(gs):
                for g in gs:
                    jb = 4 * J + g
                    eh = e_cur[g // 2][:, (g % 2) * 512:(g % 2 + 1) * 512]
                    for ct in range(2):
                        nc.tensor.matmul(
                            u_t[ct][:, :],
                            lhsT=vT_sb[:, jb * C + ct * 128:jb * C + ct * 128 + 128],
                            rhs=eh,
                            start=(jb == 0),
                            stop=(jb == 15),
                        )

            def emit_z(gs):
                for g in gs:
                    eh = e_cur[g // 2][:, (g % 2) * 512:(g % 2 + 1) * 512]
                    nc.tensor.matmul(
                        zpart[32 * g:32 * (g + 1), :],
                        lhsT=ones_sb,
                        rhs=eh,
                        start=(J == 0),
                        stop=(J == 3),
                        tile_position=(0, 32 * g),
                        # 4 col-groups accumulate into disjoint 32-row bands of
                        # one bank; per-element has_written makes this safe on
                        # HW but the sim's group map is partition-blind.
                        skip_group_check=True,
                    )

            if k < 15:
                emit_u(range(4))
                # the 4 Z matmuls back-to-back so they pack into one
                # concurrent column-tiled group (~1 matmul of PE time for 4)
                emit_z(range(4))
            else:
                # last block: Z ahead of the second U half-stream (same exp
                # gate) so zpart completes ~0.9us before the U stream ends
                # and the final tail chain overlaps the last U matmuls.
                emit_u((0, 1))
                emit_z((0, 1))
                emit_z((2, 3))
                emit_u((2, 3))
            e_cur = e_next
            if pending is not None:
                # previous quarter's tail (pending is set at J==3, consumed at
                # the next block = J==0 of quarter qd), now safely behind this
                # block's score+U matmuls in queue order
                emit_tail_b((qd - 1) * 512, pending, on_dve=(qd - 1 >= 2))
                pending = None
            if J == 3:
                pending = emit_tail_a(u_t, zpart, last=(k == 15))
                if k == 15:
                    emit_tail_b(i0, pending, on_dve=True, last=True)
                    pending = None

    nc.compile()
    return nc


def get_nc():
    if "nc" not in _cache:
        _cache["nc"] = _build_nc()
    return _cache["nc"]


def make_in_maps(x, Wq, bq, Wk, bk, Wv, bv, gamma):
    import ml_dtypes

    bf = ml_dtypes.bfloat16
    x = np.asarray(x, dtype=np.float32)
    g = float(np.asarray(gamma, np.float32).reshape(-1)[0])
    cpack = np.zeros((128, 257), np.float32)
    cpack[0::32, 0] = 1.0                       # indicator
    cpack[:, 1:129] = g * np.eye(128)           # gamma * I
    cpack[:, 129:257] = 1.0                     # ones
    # bv folds into the residual: out = gamma*(Wv x E^T)/Z + (x + gamma*bv)
    xr_bias = (g * np.asarray(bv, np.float32)).reshape(1, C, 1)
    shared = {
        "wpack": np.ascontiguousarray(
            np.concatenate([np.tile(np.asarray(Wq, np.float32).T, (1, 4)),
                            np.tile(np.asarray(Wk, np.float32).T, (1, 4)),
                            np.asarray(Wv, np.float32).T],
                           axis=1)).astype(bf),
        "bqk": np.stack([np.tile(np.asarray(bq, np.float32), 4),
                         np.tile(np.asarray(bk, np.float32), 4)], axis=1),
        "cpack": cpack.astype(bf),
    }
    xrs = (x + xr_bias).astype(np.float32)
    return [
        dict(shared, xr=np.ascontiguousarray(xrs[b]),
             xb=np.ascontiguousarray(x[b]).astype(bf))
        for b in range(B)
    ]


def kernel(x, Wq, bq, Wk, bk, Wv, bv, gamma):
    from concourse.bass_utils import run_bass_kernel_spmd

    nc = get_nc()
    in_maps = make_in_maps(x, Wq, bq, Wk, bk, Wv, bv, gamma)
    res = run_bass_kernel_spmd(nc, in_maps, list(range(NCORES)))
    return np.stack([res.results[b]["y"] for b in range(B)], axis=0)


# revision 29
# speedup vs baseline: 1.2167x; 1.0301x over previous
"""Trainium2 Bass kernel for nn_AttentionModule (B=8, C=256, L=2048, D=32).

Per-batch computation (data-parallel: one batch per NeuronCore, 8 cores):
    qT = Wq @ x + bq            # (D, L)
    kT = Wk @ x + bk            # (D, L)
    vT = x.T @ Wv.T             # (L, C)   -- bias bv handled algebraically:
                                #   out = v @ attn^T = Wv x E^T/Z + bv (rows of
                                #   attn sum to 1), so bv folds into the
                                #   residual as xr = x + gamma*bv host-side.
    ST = kT.T @ qT              # (L_j, L_i) = S[i,j] transposed
    E  = exp(ST)                # no max-subtraction: max|S| ~ 46, exp fits fp32
    Z  = sum_j E[j, i]          # column-tiled PE ones-matmuls -> PSUM partials
    U  = vT.T @ E               # (C, L_i)
    y  = gamma * U / Z + xr

Block (qd, J) covers j-blocks {4J+g} (J-major), so the first score matmuls
need only k-projection chunk 0 instead of all four: the exp stream starts
right after the first 512 columns of x land, ~4us earlier than a g-major
mapping. kT4's strip layout absorbs this via a diagonal gather in the k-bias
copyback (strip g of col-block J = replica g, cols 128g of projection chunk
J).

Engine split: PE does all matmuls including the Z partial reduction (4
column-tiled concurrent ones-matmuls per block accumulating into one
PSUM bank), the final Z reduce + transpose (so the reciprocal runs
128-wide) and the 1/Z broadcast (one K=1 N=512 matmul), all with bf16
stationary data (1 cyc/row). ACT does exp (the pacing producer: ~1.1us per
[128,1024] tile), half the k-bias copybacks, 4 vT casts, and the last
quarter's tail copies. DVE does the q-bias copyback, 12 vT casts, Z/U/rd
copybacks, reciprocal, and the last two quarters' y finalize; GPSIMD
finalizes quarters 0-1 (hidden under later compute).

Schedule: 16 (quarter, J) blocks are software-pipelined - scores+exp
for block k+1 are emitted before block k's U/Z matmuls, and each
quarter's tail matmuls are deferred into the next quarter's first block
so they can never head-of-line block the score pipeline on the PE
queue. Block 15 instead splits its Z matmuls between the two U half-streams
and inlines the tail, so the final Z->1/Z->broadcast->y chain starts ~1us
earlier and nothing idles behind it. Wq/Wk are replicated 4x host-side so
the projections emit qT4 (replicated strips) and kT4 directly. xb streams
in column chunks; ~20 dummy matmuls on a memset scratch tile (no DMA
dependency) warm the PE's HAM clock gate to 2.4GHz before the projections.
bv folds into the residual (attention rows sum to 1) and gamma into the
transpose identity, so no separate bias/scale passes exist anywhere.
"""

import numpy as np

B, C, L, D = 8, 256, 2048, 32
NCORES = 8

_cache = {}


def _build_nc():
    from contextlib import ExitStack

    import concourse.bacc as bacc
    import concourse.tile as tile
    from concourse import mybir

    f32 = mybir.dt.float32
    bf16 = mybir.dt.bfloat16
    EXP = mybir.ActivationFunctionType.Exp
    IDENT = mybir.ActivationFunctionType.Identity

    nc = bacc.Bacc("TRN2", target_bir_lowering=False, debug=False)

    xr_d = nc.dram_tensor("xr", [C, L], f32, kind="ExternalInput")
    xb_d = nc.dram_tensor("xb", [C, L], bf16, kind="ExternalInput")
    wpack_d = nc.dram_tensor("wpack", [C, 512], bf16, kind="ExternalInput")
    bqk_d = nc.dram_tensor("bqk", [128, 2], f32, kind="ExternalInput")
    # consts packed: col 0 = indicator (1 at rows 0,32,64,96), cols 1:129 =
    # gamma*I, cols 129:257 = ones (Z matmul lhsT uses 32 cols;
    # row 0 doubles as the broadcast lhsT onesr).
    cpack_d = nc.dram_tensor("cpack", [128, 257], bf16, kind="ExternalInput")
    y_d = nc.dram_tensor("y", [C, L], f32, kind="ExternalOutput")

    xr_ap = xr_d.ap()
    y_ap = y_d.ap()

    with tile.TileContext(nc) as tc, ExitStack() as ctx:
        singles = ctx.enter_context(tc.tile_pool(name="singles", bufs=1))
        big = ctx.enter_context(tc.tile_pool(name="big", bufs=1))
        ps = ctx.enter_context(tc.tile_pool(name="ps", bufs=2, space="PSUM"))
        up = ctx.enter_context(tc.tile_pool(name="up", bufs=1, space="PSUM"))
        zp = ctx.enter_context(tc.tile_pool(name="zp", bufs=2, space="PSUM"))
        epool = ctx.enter_context(tc.tile_pool(name="epool", bufs=10))
        ypool = ctx.enter_context(tc.tile_pool(name="ypool", bufs=4))
        uspool = ctx.enter_context(tc.tile_pool(name="uspool", bufs=2))
        rpool = ctx.enter_context(tc.tile_pool(name="rpool", bufs=2))

        # scratch for the HAM warmup: memset (no DMA dependency) so the
        # dummy matmuls start right after the framework preamble.
        scr_sb = singles.tile([128, 256], bf16, tag="scr")
        nc.gpsimd.memset(scr_sb[:], 0.0)

        # ---- loads: critical tensors first; the two HW DMA queues share the
        # HBM domain with the 7 sibling cores (~116GB/s effective), so what
        # matters is byte order, not queue count. First projection chunk
        # needs wpack (q/k cols) + xb cols 0:1024 of both channel chunks.
        wpack_sb, xb_sb = [], []
        for ct in range(2):
            tw = singles.tile([128, 512], bf16, tag=f"wp{ct}", name=f"wp{ct}")
            nc.sync.dma_start(out=tw[:], in_=wpack_d.ap()[ct * 128:(ct + 1) * 128, :])
            wpack_sb.append(tw)
        cpack_sb = singles.tile([128, 257], bf16, tag="cpack")
        nc.scalar.dma_start(out=cpack_sb[:], in_=cpack_d.ap()[:, :])
        bqk_sb = singles.tile([128, 2], f32, tag="bqk")
        nc.scalar.dma_start(out=bqk_sb[:], in_=bqk_d.ap()[:, :])
        for ct in range(2):
            xb_sb.append(big.tile([128, L], bf16, tag=f"xb{ct}", name=f"xb{ct}"))
        for half in range(2):
            for ct in range(2):
                nc.sync.dma_start(
                    out=xb_sb[ct][:, half * 1024:(half + 1) * 1024],
                    in_=xb_d.ap()[ct * 128:(ct + 1) * 128, half * 1024:(half + 1) * 1024],
                )
        indq_sb = cpack_sb[:, 0:1]
        identg_sb = cpack_sb[:, 1:129]
        ones_sb = cpack_sb[:, 129:161]
        onesr_sb = cpack_sb[0:1, 129:257]

        # ~4us of dummy matmuls on the scratch tile while the DMAs stream in:
        # pushes the PE's HAM clock-gate to full speed so the projections
        # run at 2.4GHz instead of 1.2.
        warm = ps.tile([128, 256], f32, tag="ps", name="warm")
        for w in range(16):
            nc.tensor.matmul(
                warm[:, :],
                lhsT=scr_sb[:, 0:128],
                rhs=scr_sb[:, 0:256],
                start=True,
                stop=True,
            )

        # ---- projections (PSUM borrowed from the U-accumulator pool) ----
        # Wq/Wk are replicated 4x host-side (wqk = [C, 128+128]) so the
        # projection matmuls directly produce qT replicated across the four
        # 32-partition strips (qT4) and kT in per-strip j-block layout (kT4):
        # no SBUF->SBUF replicate/rearrange DMAs at all.
        qT4 = big.tile([128, L], bf16, tag="qT4")
        # kT4: strip g at col-block J holds j-block jb = 4J+g (J-major), i.e.
        # rows 128g:128g+128 of projection chunk J; written by a 4-way
        # diagonal gather from the replicated k-projection PSUM.
        kT4 = big.tile([128, 512], bf16, tag="kT4")

        # the q/k projection chunks spread over all four PSUM chains (up x2,
        # zp x2) as their FIRST links, so the four chunk pipelines run in
        # parallel instead of ping-ponging through two buffers.
        def proj_psum(pool, name):
            if pool == "u0" or pool == "u1":
                return up.tile([128, 512], f32, tag=pool, name=name, bufs=1)
            if pool == "z":
                return zp.tile([128, 512], f32, tag="z", name=name)
            return ps.tile([128, 512], f32, tag="ps", name=name)

        def emit_s_exp(k):
            qd, J = divmod(k, 4)
            i0 = qd * 512
            e_tiles = []
            for pair in range(2):
                stp = ps.tile([128, 1024], f32, tag="ps")
                for h in range(2):
                    g = 2 * pair + h
                    nc.tensor.matmul(
                        stp[:, h * 512:(h + 1) * 512],
                        lhsT=kT4[32 * g:32 * (g + 1), J * 128:(J + 1) * 128],
                        rhs=qT4[32 * g:32 * (g + 1), i0:i0 + 512],
                        start=True,
                        stop=True,
                        tile_position=(32 * g, 0),
                    )
                e2 = epool.tile([128, 1024], bf16, tag="e")
                nc.scalar.activation(e2[:], stp[:], EXP)
                e_tiles.append(e2)
            return e_tiles

        # vT[j, c] stored as [128, 16*256]: block jb holds vT[jb*128 + p, c].
        # v-projection chains use u0/u1/z PSUM only (never the stp-shared
        # 'ps' slots, except the last two blocks emitted after all stp
        # hoists) so interleaving v-projection with score hoists can't
        # couple a v-cast chain to the exp stream. Block (qd, J) consumes
        # lb in {4J..4J+3}; VP_A (blocks 0-7, needs only the first half of
        # x) fills the PE while the second half of x is still in flight.
        vT_sb = big.tile([128, 16 * C], bf16, tag="vT")
        VP_POOL = {0: "u0", 4: "u0", 8: "u0", 1: "u1", 5: "u1", 9: "u1",
                   2: "z", 3: "z", 6: "z", 7: "z", 10: "z", 11: "z",
                   12: "z", 13: "z", 14: "ps", 15: "ps"}
        VP_A = [2, 0, 1, 3, 4, 5, 6, 7]
        VP_B = [8, 9, 10, 11, 12, 13, 14, 15]

        def emit_vproj(lbs):
            # casts alternate DVE/ACT so eight in a row can't starve the
            # q-bias and k-cast ops that gate the hoisted score blocks
            for n, lb in enumerate(lbs):
                p = proj_psum(VP_POOL[lb], f"pv{lb}")
                for ct in range(2):
                    nc.tensor.matmul(
                        p[:, :C],
                        lhsT=xb_sb[ct][:, lb * 128:(lb + 1) * 128],
                        rhs=wpack_sb[ct][:, 256:512],
                        start=(ct == 0),
                        stop=(ct == 1),
                    )
                if n % 2 == 0:
                    nc.vector.tensor_copy(vT_sb[:, lb * C:(lb + 1) * C], p[:, :C])
                else:
                    nc.scalar.copy(vT_sb[:, lb * C:(lb + 1) * C], p[:, :C])

        # scores + exp for block k are emitted right after projection chunk k
        # (J-major block map: block (0, J) needs only kT4 col-block J = chunk
        # J and qT4 quarter 0 = chunk 0), so the exp stream starts as soon as
        # the first 512 columns of x are projected instead of after all four
        # chunks + the v-projection.
        hoisted_e = []
        PQ_POOL = {0: "u0", 1: "ps", 2: "z", 3: "z"}
        PK_POOL = {0: "u1", 1: "z", 2: "z", 3: "ps"}
        # The k bias bk is dropped entirely: S[j,i] = q_i . (k_j + bk) adds
        # q_i . bk, constant over j, which the softmax cancels (it scales E
        # and Z by the same e^{q.bk} ~ e^{+-0.3} per column).
        # k-projection: 4 column-tiled CONCURRENT matmuls per chunk, replica
        # strip g streaming its own 128-col slice of x, so the PSUM lands
        # directly in kT4's diagonal (strip g = j-block 4*it+g) layout AND
        # the k-proj streams 256 cols/chunk instead of 1024.
        for it in range(4):
            pq = proj_psum(PQ_POOL[it], f"pq{it}")
            pk = proj_psum(PK_POOL[it], f"pk{it}")
            for ct in range(2):
                nc.tensor.matmul(
                    pq[:, :],
                    lhsT=wpack_sb[ct][:, 0:128],
                    rhs=xb_sb[ct][:, it * 512:(it + 1) * 512],
                    start=(ct == 0),
                    stop=(ct == 1),
                )
            for ct in range(2):
                for g in range(4):
                    nc.tensor.matmul(
                        pk[32 * g:32 * (g + 1), 0:128],
                        lhsT=wpack_sb[ct][:, 128 + 32 * g:128 + 32 * (g + 1)],
                        rhs=xb_sb[ct][:, it * 512 + 128 * g:it * 512 + 128 * (g + 1)],
                        start=(ct == 0),
                        stop=(ct == 1),
                        tile_position=(0, 32 * g),
                        skip_group_check=True,
                    )
            nc.vector.tensor_scalar_add(
                qT4[:, it * 512:(it + 1) * 512], pq[:, :], bqk_sb[:, 0:1]
            )
            # kT4 cast on ACT for even chunks so chunk 0's cast runs in
            # parallel with its q-bias on DVE (S(0) gates on both)
            if it % 2 == 0:
                nc.scalar.copy(kT4[:, 128 * it:128 * (it + 1)], pk[:, 0:128])
            else:
                nc.vector.tensor_copy(kT4[:, 128 * it:128 * (it + 1)], pk[:, 0:128])
            hoisted_e.append(emit_s_exp(it))
            if it == 1:
                emit_vproj(VP_A)

        emit_vproj(VP_B)

        # fp32 residual xr = x + gamma*bv (prepared host-side)
        xr_sb = []
        for ct in range(2):
            t = big.tile([128, L], f32, tag=f"x{ct}")
            nc.sync.dma_start(out=t[:], in_=xr_ap[ct * 128:(ct + 1) * 128, :])
            xr_sb.append(t)

        # ---- attention, processed in i-quarters of 512 columns ----
        # emit_tail: reduce the 4 Z partial rows, 1/Z, transpose+broadcast via
        # PE, then y = us*(gamma/Z) + xr. Quarters 0-1 finalize on GPSIMD
        # (hidden under later compute); quarters 2-3 on DVE straight from PSUM.
        def emit_tail_a(u_t, zpart, last=False):
            # phase A (at J==3): free the U banks, copy Z partials to SBUF,
            # and allocate the tail PSUM tiles (keeping the zp pool rotation:
            # zpart, zt, rd_ps, rb_ps per quarter).
            zsb = rpool.tile([128, 512], bf16, tag="zsb", name="zsb")
            if last:
                # the final chain is pure latency: zsb halves go FIRST on
                # both ACT+DVE (the zt matmuls gate on it), us copies after
                # (their consumer, the y multiply, comes ~2us later)
                nc.scalar.copy(zsb[0:64, :], zpart[0:64, :])
                nc.vector.tensor_copy(zsb[64:128, :], zpart[64:128, :])
            us = []
            for ct in range(2):
                u = uspool.tile([128, 512], f32, tag=f"us{ct}", name=f"us{ct}")
                if last and ct == 0:
                    nc.scalar.copy(u[:], u_t[ct][:, :])
                else:
                    nc.vector.tensor_copy(u[:], u_t[ct][:, :])
                us.append(u)
            if not last:
                nc.vector.tensor_copy(zsb[:], zpart[:, :])
            # allocation order is load-bearing: with [rd_ps, zt, rb_ps] the
            # NEXT quarter's zpart inherits zt's bank (freed by the reciprocal
            # ~1.1us into this tail) instead of rd_ps's (freed by the rd cast
            # ~2.3us in), so the new quarter's first Z matmuls never stall on
            # this quarter's tail chain.
            # the last quarter uses column-tiled concurrent rd matmuls whose
            # chunks land at psum partitions 32c, so its rd_ps spans all
            # partitions; mid-kernel quarters keep the [1,512] shape that
            # preserves the zp bank rotation.
            rd_ps = zp.tile([128 if last else 1, 512], f32, tag="z", name="rd_ps")
            zt = zp.tile([128, 4], f32, tag="z", name="zt")
            rb_ps = zp.tile([128, 512], f32, tag="z", name="rb_ps")
            return (us, zsb, zt, rd_ps, rb_ps)

        def emit_tail_b(i0, pend, on_dve, last=False):
            # phase B (emitted after the NEXT block's U stream so none of
            # these PE matmuls can head-of-line block the score pipeline):
            # Z reduce+transpose, 1/Z, broadcast, y = us*(gamma/Z) + xr.
            us, zsb, zt, rd_ps, rb_ps = pend
            for c in range(4):
                nc.tensor.matmul(
                    zt[:, c:c + 1],
                    lhsT=zsb[:, 128 * c:128 * (c + 1)],
                    rhs=indq_sb,
                    start=True,
                    stop=True,
                )
            rt = rpool.tile([128, 4], bf16, tag="rt", name="rt")
            with nc.allow_low_precision(reason="1/Z in bf16: 0.4% on a 2e-2 budget"):
                nc.vector.reciprocal(rt[:], zt[:, 0:4])
            rd = rpool.tile([1, 512], bf16, tag="rd", name="rd")
            if last:
                # 4 column-tiled CONCURRENT rd matmuls (chunk c at psum
                # partition 32c) + 4 small casts split DVE/ACT: ~1us off the
                # pure-latency final chain.
                for c in range(4):
                    nc.tensor.matmul(
                        rd_ps[32 * c:32 * c + 1, 128 * c:128 * (c + 1)],
                        lhsT=rt[:, c:c + 1],
                        rhs=identg_sb,
                        start=True,
                        stop=True,
                        tile_position=(0, 32 * c),
                    )
                for c in range(4):
                    src = rd_ps[32 * c:32 * c + 1, 128 * c:128 * (c + 1)]
                    dst = rd[0:1, 128 * c:128 * (c + 1)]
                    if c % 2 == 0:
                        nc.vector.tensor_copy(dst, src)
                    else:
                        nc.scalar.copy(dst, src)
            else:
                for c in range(4):
                    nc.tensor.matmul(
                        rd_ps[0:1, 128 * c:128 * (c + 1)],
                        lhsT=rt[:, c:c + 1],
                        rhs=identg_sb,
                        start=True,
                        stop=True,
                    )
                nc.vector.tensor_copy(rd[:], rd_ps[0:1, :])
            # single K=1 N=512 broadcast matmul (the 4-chunk split buys
            # nothing once rd exists as one row)
            nc.tensor.matmul(
                rb_ps[:, :],
                lhsT=onesr_sb,
                rhs=rd[0:1, :],
                start=True,
                stop=True,
            )
            if not on_dve:
                rb_sb = rpool.tile([128, 512], f32, tag="rb", name="rb_sb")
                nc.vector.tensor_copy(rb_sb[:], rb_ps[:, :])
            eng = nc.vector if on_dve else nc.gpsimd
            for ct in range(2):
                yt = ypool.tile([128, 512], f32, tag="y", name="yt")
                eng.tensor_mul(yt[:], us[ct][:], rb_ps[:, :] if on_dve else rb_sb[:])
                eng.tensor_add(yt[:], yt[:], xr_sb[ct][:, i0:i0 + 512])
                nc.sync.dma_start(
                    out=y_ap[ct * 128:(ct + 1) * 128, i0:i0 + 512], in_=yt[:]
                )

        # Software pipeline over 16 (quarter, J) blocks: the scores + exp for
        # block k+1 are emitted at high priority BEFORE block k's U/Z matmuls,
        # so the next block's scores can never be head-of-line blocked behind
        # this block's U stream or the quarter tail on the PE queue.
        u_t = None
        zpart = None
        pending = None
        e_cur = hoisted_e[0]
        for k in range(16):
            qd, J = divmod(k, 4)
            i0 = qd * 512
            if J == 0:
                u_t = [
                    up.tile([128, 512], f32, tag=f"u{ct}", name=f"u{ct}", bufs=1)
                    for ct in range(2)
                ]
                # Z partials: row band 32g accumulates sum over J of E[4J+g];
                # written by column-tiled concurrent ones-matmuls.
                zpart = zp.tile([128, 512], f32, tag="z", name="zpart")
            if k + 1 < len(hoisted_e):
                e_next = hoisted_e[k + 1]
            else:
                e_next = emit_s_exp(k + 1) if k < 15 else None

            def emit_u(gs):
                for g in gs:
                    jb = 4 * J + g
                    eh = e_cur[g // 2][:, (g % 2) * 512:(g % 2 + 1) * 512]
                    for ct in range(2):
                        nc.tensor.matmul(
                            u_t[ct][:, :],
                            lhsT=vT_sb[:, jb * C + ct * 128:jb * C + ct * 128 + 128],
                            rhs=eh,
                            start=(jb == 0),
                            stop=(jb == 15),
                        )

            def emit_z(gs):
                for g in gs:
                    eh = e_cur[g // 2][:, (g % 2) * 512:(g % 2 + 1) * 512]
                    nc.tensor.matmul(
                        zpart[32 * g:32 * (g + 1), :],
                        lhsT=ones_sb,
                        rhs=eh,
                        start=(J == 0),
                        stop=(J == 3),
                        tile_position=(0, 32 * g),
                        # 4 col-groups accumulate into disjoint 32-row bands of
                        # one bank; per-element has_written makes this safe on
                        # HW but the sim's group map is partition-blind.
                        skip_group_check=True,
                    )

            if k < 15:
                emit_u(range(4))
                # the 4 Z matmuls back-to-back so they pack into one
                # concurrent column-tiled group (~1 matmul of PE time for 4)
                emit_z(range(4))
            else:
                # last block: Z ahead of the second U half-stream (same exp
                # gate) so zpart completes ~0.9us before the U stream ends
                # and the final tail chain overlaps the last U matmuls.
                emit_u((0, 1))
                emit_z((0, 1))
                emit_z((2, 3))
                emit_u((2, 3))
            e_cur = e_next
            if pending is not None:
                # previous quarter's tail (pending is set at J==3, consumed at
                # the next block = J==0 of quarter qd), now safely behind this
                # block's score+U matmuls in queue order
                emit_tail_b((qd - 1) * 512, pending, on_dve=(qd - 1 >= 2))
                pending = None
            if J == 3:
                pending = emit_tail_a(u_t, zpart, last=(k == 15))
                if k == 15:
                    emit_tail_b(i0, pending, on_dve=True, last=True)
                    pending = None

    nc.compile()
    return nc


def get_nc():
    if "nc" not in _cache:
        _cache["nc"] = _build_nc()
    return _cache["nc"]


def make_in_maps(x, Wq, bq, Wk, bk, Wv, bv, gamma):
    import ml_dtypes

    bf = ml_dtypes.bfloat16
    x = np.asarray(x, dtype=np.float32)
    g = float(np.asarray(gamma, np.float32).reshape(-1)[0])
    cpack = np.zeros((128, 257), np.float32)
    cpack[0::32, 0] = 1.0                       # indicator
    cpack[:, 1:129] = g * np.eye(128)           # gamma * I
    cpack[:, 129:257] = 1.0                     # ones
    # bv folds into the residual: out = gamma*(Wv x E^T)/Z + (x + gamma*bv)
    xr_bias = (g * np.asarray(bv, np.float32)).reshape(1, C, 1)
    shared = {
        "wpack": np.ascontiguousarray(
            np.concatenate([np.tile(np.asarray(Wq, np.float32).T, (1, 4)),
                            np.tile(np.asarray(Wk, np.float32).T, (1, 4)),
                            np.asarray(Wv, np.float32).T],
                           axis=1)).astype(bf),
        "bqk": np.stack([np.tile(np.asarray(bq, np.float32), 4),
                         np.tile(np.asarray(bk, np.float32), 4)], axis=1),
        "cpack": cpack.astype(bf),
    }
    xrs = (x + xr_bias).astype(np.float32)
    return [
        dict(shared, xr=np.ascontiguousarray(xrs[b]),
             xb=np.ascontiguousarray(x[b]).astype(bf))
        for b in range(B)
    ]


def kernel(x, Wq, bq, Wk, bk, Wv, bv, gamma):
    from concourse.bass_utils import run_bass_kernel_spmd

    nc = get_nc()
    in_maps = make_in_maps(x, Wq, bq, Wk, bk, Wv, bv, gamma)
    res = run_bass_kernel_spmd(nc, in_maps, list(range(NCORES)))
    return np.stack([res.results[b]["y"] for b in range(B)], axis=0)


# revision 33
# speedup vs baseline: 1.2427x; 1.0214x over previous
"""Trainium2 Bass kernel for nn_AttentionModule (B=8, C=256, L=2048, D=32).

Per-batch computation (data-parallel: one batch per NeuronCore, 8 cores):
    qT = Wq @ x + bq            # (D, L)
    kT = Wk @ x + bk            # (D, L)
    vT = x.T @ Wv.T             # (L, C)   -- bias bv handled algebraically:
                                #   out = v @ attn^T = Wv x E^T/Z + bv (rows of
                                #   attn sum to 1), so bv folds into the
                                #   residual as xr = x + gamma*bv host-side.
    ST = kT.T @ qT              # (L_j, L_i) = S[i,j] transposed
    E  = exp(ST)                # no max-subtraction: max|S| ~ 46, exp fits fp32
    Z  = sum_j E[j, i]          # column-tiled PE ones-matmuls -> PSUM partials
    U  = vT.T @ E               # (C, L_i)
    y  = gamma * U / Z + xr

Block (qd, J) covers j-blocks {4J+g} (J-major), so the first score matmuls
need only k-projection chunk 0 instead of all four: the exp stream starts
right after the first 512 columns of x land, ~4us earlier than a g-major
mapping. kT4's strip layout absorbs this via a diagonal gather in the k-bias
copyback (strip g of col-block J = replica g, cols 128g of projection chunk
J).

Engine split: PE does all matmuls including the Z partial reduction (4
column-tiled concurrent ones-matmuls per block accumulating into one
PSUM bank), the final Z reduce + transpose (so the reciprocal runs
128-wide) and the 1/Z broadcast (one K=1 N=512 matmul), all with bf16
stationary data (1 cyc/row). ACT does exp (the pacing producer: ~1.1us per
[128,1024] tile), half the k-bias copybacks, 4 vT casts, and the last
quarter's tail copies. DVE does the q-bias copyback, 12 vT casts, Z/U/rd
copybacks, reciprocal, and the last two quarters' y finalize; GPSIMD
finalizes quarters 0-1 (hidden under later compute).

Schedule: 16 (quarter, J) blocks are software-pipelined - scores+exp
for block k+1 are emitted before block k's U/Z matmuls, and each
quarter's tail matmuls are deferred into the next quarter's first block
so they can never head-of-line block the score pipeline on the PE
queue. Block 15 instead splits its Z matmuls between the two U half-streams
and inlines the tail, so the final Z->1/Z->broadcast->y chain starts ~1us
earlier and nothing idles behind it. Wq/Wk are replicated 4x host-side so
the projections emit qT4 (replicated strips) and kT4 directly. xb streams
in column chunks; ~20 dummy matmuls on a memset scratch tile (no DMA
dependency) warm the PE's HAM clock gate to 2.4GHz before the projections.
bv folds into the residual (attention rows sum to 1) and gamma into the
transpose identity, so no separate bias/scale passes exist anywhere.
"""

import numpy as np

B, C, L, D = 8, 256, 2048, 32
NCORES = 8

_cache = {}


def _build_nc():
    from contextlib import ExitStack

    import concourse.bacc as bacc
    import concourse.tile as tile
    from concourse import mybir

    f32 = mybir.dt.float32
    bf16 = mybir.dt.bfloat16
    EXP = mybir.ActivationFunctionType.Exp
    IDENT = mybir.ActivationFunctionType.Identity

    nc = bacc.Bacc("TRN2", target_bir_lowering=False, debug=False)

    xr_d = nc.dram_tensor("xr", [C, L], f32, kind="ExternalInput")
    xb_d = nc.dram_tensor("xb", [C, L], bf16, kind="ExternalInput")
    wpack_d = nc.dram_tensor("wpack", [C, 512], bf16, kind="ExternalInput")
    bqk_d = nc.dram_tensor("bqk", [128, 2], f32, kind="ExternalInput")
    # consts packed: col 0 = indicator (1 at rows 0,32,64,96), cols 1:129 =
    # gamma*I, cols 129:257 = ones (Z matmul lhsT uses 32 cols;
    # row 0 doubles as the broadcast lhsT onesr).
    cpack_d = nc.dram_tensor("cpack", [128, 257], bf16, kind="ExternalInput")
    y_d = nc.dram_tensor("y", [C, L], f32, kind="ExternalOutput")

    xr_ap = xr_d.ap()
    y_ap = y_d.ap()

    with tile.TileContext(nc) as tc, ExitStack() as ctx:
        singles = ctx.enter_context(tc.tile_pool(name="singles", bufs=1))
        big = ctx.enter_context(tc.tile_pool(name="big", bufs=1))
        ps = ctx.enter_context(tc.tile_pool(name="ps", bufs=2, space="PSUM"))
        up = ctx.enter_context(tc.tile_pool(name="up", bufs=1, space="PSUM"))
        zp = ctx.enter_context(tc.tile_pool(name="zp", bufs=2, space="PSUM"))
        epool = ctx.enter_context(tc.tile_pool(name="epool", bufs=10))
        ypool = ctx.enter_context(tc.tile_pool(name="ypool", bufs=4))
        uspool = ctx.enter_context(tc.tile_pool(name="uspool", bufs=2))
        rpool = ctx.enter_context(tc.tile_pool(name="rpool", bufs=2))

        # scratch for the HAM warmup: memset (no DMA dependency) so the
        # dummy matmuls start right after the framework preamble.
        scr_sb = singles.tile([128, 256], bf16, tag="scr")
        nc.gpsimd.memset(scr_sb[:], 0.0)

        # ---- loads: critical tensors first; the two HW DMA queues share the
        # HBM domain with the 7 sibling cores (~116GB/s effective), so what
        # matters is byte order, not queue count. First projection chunk
        # needs wpack (q/k cols) + xb cols 0:1024 of both channel chunks.
        wpack_sb, xb_sb = [], []
        for ct in range(2):
            tw = singles.tile([128, 512], bf16, tag=f"wp{ct}", name=f"wp{ct}")
            nc.sync.dma_start(out=tw[:], in_=wpack_d.ap()[ct * 128:(ct + 1) * 128, :])
            wpack_sb.append(tw)
        cpack_sb = singles.tile([128, 257], bf16, tag="cpack")
        nc.scalar.dma_start(out=cpack_sb[:], in_=cpack_d.ap()[:, :])
        bqk_sb = singles.tile([128, 2], f32, tag="bqk")
        nc.scalar.dma_start(out=bqk_sb[:], in_=bqk_d.ap()[:, :])
        for ct in range(2):
            xb_sb.append(big.tile([128, L], bf16, tag=f"xb{ct}", name=f"xb{ct}"))
        for half in range(2):
            for ct in range(2):
                nc.sync.dma_start(
                    out=xb_sb[ct][:, half * 1024:(half + 1) * 1024],
                    in_=xb_d.ap()[ct * 128:(ct + 1) * 128, half * 1024:(half + 1) * 1024],
                )
        indq_sb = cpack_sb[:, 0:1]
        identg_sb = cpack_sb[:, 1:129]
        ones_sb = cpack_sb[:, 129:161]
        onesr_sb = cpack_sb[0:1, 129:257]

        # ~4us of dummy matmuls on the scratch tile while the DMAs stream in:
        # pushes the PE's HAM clock-gate to full speed so the projections
        # run at 2.4GHz instead of 1.2.
        warm = ps.tile([128, 256], f32, tag="ps", name="warm")
        for w in range(16):
            nc.tensor.matmul(
                warm[:, :],
                lhsT=scr_sb[:, 0:128],
                rhs=scr_sb[:, 0:256],
                start=True,
                stop=True,
            )

        # ---- projections (PSUM borrowed from the U-accumulator pool) ----
        # Wq/Wk are replicated 4x host-side (wqk = [C, 128+128]) so the
        # projection matmuls directly produce qT replicated across the four
        # 32-partition strips (qT4) and kT in per-strip j-block layout (kT4):
        # no SBUF->SBUF replicate/rearrange DMAs at all.
        qT4 = big.tile([128, L], bf16, tag="qT4")
        # kT4: strip g at col-block J holds j-block jb = 4J+g (J-major), i.e.
        # rows 128g:128g+128 of projection chunk J; written by a 4-way
        # diagonal gather from the replicated k-projection PSUM.
        kT4 = big.tile([128, 512], bf16, tag="kT4")

        # the q/k projection chunks spread over all four PSUM chains (up x2,
        # zp x2) as their FIRST links, so the four chunk pipelines run in
        # parallel instead of ping-ponging through two buffers.
        def proj_psum(pool, name):
            if pool == "u0" or pool == "u1":
                return up.tile([128, 512], f32, tag=pool, name=name, bufs=1)
            if pool == "z":
                return zp.tile([128, 512], f32, tag="z", name=name)
            return ps.tile([128, 512], f32, tag="ps", name=name)

        def emit_s_exp(k):
            qd, J = divmod(k, 4)
            i0 = qd * 512
            e_tiles = []
            for pair in range(2):
                stp = ps.tile([128, 1024], f32, tag="ps")
                for h in range(2):
                    g = 2 * pair + h
                    nc.tensor.matmul(
                        stp[:, h * 512:(h + 1) * 512],
                        lhsT=kT4[32 * g:32 * (g + 1), J * 128:(J + 1) * 128],
                        rhs=qT4[32 * g:32 * (g + 1), i0:i0 + 512],
                        start=True,
                        stop=True,
                        tile_position=(32 * g, 0),
                    )
                e2 = epool.tile([128, 1024], bf16, tag="e")
                nc.scalar.activation(e2[:], stp[:], EXP)
                e_tiles.append(e2)
            return e_tiles

        # vT[j, c] stored as [128, 16*256]: block jb holds vT[jb*128 + p, c].
        # v-projection chains use u0/u1/z PSUM only (never the stp-shared
        # 'ps' slots, except the last two blocks emitted after all stp
        # hoists) so interleaving v-projection with score hoists can't
        # couple a v-cast chain to the exp stream. Block (qd, J) consumes
        # lb in {4J..4J+3}; VP_A (blocks 0-7, needs only the first half of
        # x) fills the PE while the second half of x is still in flight.
        vT_sb = big.tile([128, 16 * C], bf16, tag="vT")
        VP_POOL = {0: "u0", 4: "u0", 8: "u0", 1: "u1", 5: "u1", 9: "u1",
                   2: "z", 3: "z", 6: "z", 7: "z", 10: "z", 11: "z",
                   12: "z", 13: "z", 14: "ps", 15: "ps"}
        VP_A = [2, 0, 1, 3, 4, 5, 6, 7]
        VP_B = [8, 9, 10, 11, 12, 13, 14, 15]

        def emit_vproj(lbs):
            # casts alternate DVE/ACT so eight in a row can't starve the
            # q-bias and k-cast ops that gate the hoisted score blocks
            for n, lb in enumerate(lbs):
                p = proj_psum(VP_POOL[lb], f"pv{lb}")
                for ct in range(2):
                    nc.tensor.matmul(
                        p[:, :C],
                        lhsT=xb_sb[ct][:, lb * 128:(lb + 1) * 128],
                        rhs=wpack_sb[ct][:, 256:512],
                        start=(ct == 0),
                        stop=(ct == 1),
                    )
                if n % 2 == 0:
                    nc.vector.tensor_copy(vT_sb[:, lb * C:(lb + 1) * C], p[:, :C])
                else:
                    nc.scalar.copy(vT_sb[:, lb * C:(lb + 1) * C], p[:, :C])

        # scores + exp for block k are emitted right after projection chunk k
        # (J-major block map: block (0, J) needs only kT4 col-block J = chunk
        # J and qT4 quarter 0 = chunk 0), so the exp stream starts as soon as
        # the first 512 columns of x are projected instead of after all four
        # chunks + the v-projection.
        hoisted_e = []
        PQ_POOL = {0: "u0", 1: "ps", 2: "z", 3: "z"}
        PK_POOL = {0: "u1", 1: "z", 2: "z", 3: "ps"}
        # The k bias bk is dropped entirely: S[j,i] = q_i . (k_j + bk) adds
        # q_i . bk, constant over j, which the softmax cancels (it scales E
        # and Z by the same e^{q.bk} ~ e^{+-0.3} per column).
        # k-projection: 4 column-tiled CONCURRENT matmuls per chunk, replica
        # strip g streaming its own 128-col slice of x, so the PSUM lands
        # directly in kT4's diagonal (strip g = j-block 4*it+g) layout AND
        # the k-proj streams 256 cols/chunk instead of 1024.
        for it in range(4):
            pq = proj_psum(PQ_POOL[it], f"pq{it}")
            pk = proj_psum(PK_POOL[it], f"pk{it}")
            for ct in range(2):
                nc.tensor.matmul(
                    pq[:, :],
                    lhsT=wpack_sb[ct][:, 0:128],
                    rhs=xb_sb[ct][:, it * 512:(it + 1) * 512],
                    start=(ct == 0),
                    stop=(ct == 1),
                )
            for ct in range(2):
                for g in range(4):
                    nc.tensor.matmul(
                        pk[32 * g:32 * (g + 1), 0:128],
                        lhsT=wpack_sb[ct][:, 128 + 32 * g:128 + 32 * (g + 1)],
                        rhs=xb_sb[ct][:, it * 512 + 128 * g:it * 512 + 128 * (g + 1)],
                        start=(ct == 0),
                        stop=(ct == 1),
                        tile_position=(0, 32 * g),
                        skip_group_check=True,
                    )
            nc.vector.tensor_scalar_add(
                qT4[:, it * 512:(it + 1) * 512], pq[:, :], bqk_sb[:, 0:1]
            )
            # kT4 cast on ACT for even chunks so chunk 0's cast runs in
            # parallel with its q-bias on DVE (S(0) gates on both)
            if it % 2 == 0:
                nc.scalar.copy(kT4[:, 128 * it:128 * (it + 1)], pk[:, 0:128])
            else:
                nc.vector.tensor_copy(kT4[:, 128 * it:128 * (it + 1)], pk[:, 0:128])
            hoisted_e.append(emit_s_exp(it))
            if it == 1:
                emit_vproj(VP_A)

        emit_vproj(VP_B)

        # fp32 residual xr = x + gamma*bv (prepared host-side)
        xr_sb = []
        for ct in range(2):
            t = big.tile([128, L], f32, tag=f"x{ct}")
            nc.sync.dma_start(out=t[:], in_=xr_ap[ct * 128:(ct + 1) * 128, :])
            xr_sb.append(t)

        # ---- attention, processed in i-quarters of 512 columns ----
        # emit_tail: reduce the 4 Z partial rows, 1/Z, transpose+broadcast via
        # PE, then y = us*(gamma/Z) + xr. Quarters 0-1 finalize on GPSIMD
        # (hidden under later compute); quarters 2-3 on DVE straight from PSUM.
        def emit_tail_a(u_t, zpart, last=False):
            # phase A (at J==3): free the U banks, copy Z partials to SBUF,
            # and allocate the tail PSUM tiles (keeping the zp pool rotation:
            # zpart, zt, rd_ps, rb_ps per quarter).
            # the final quarter keeps DVE free for the latency chain
            # (recip -> rd cast -> y): zsb and both us copies go to ACT,
            # which has nothing left after the last exp. Split writers to
            # one tile serialize on the tile's write history, so each copy
            # stays whole on one engine.
            zsb = rpool.tile([128, 512], bf16, tag="zsb", name="zsb")
            if last:
                nc.scalar.copy(zsb[:], zpart[:, :])
            us = []
            for ct in range(2):
                u = uspool.tile([128, 512], f32, tag=f"us{ct}", name=f"us{ct}")
                if last:
                    nc.scalar.copy(u[:], u_t[ct][:, :])
                else:
                    nc.vector.tensor_copy(u[:], u_t[ct][:, :])
                us.append(u)
            if not last:
                nc.vector.tensor_copy(zsb[:], zpart[:, :])
            # allocation order is load-bearing: with [rd_ps, zt, rb_ps] the
            # NEXT quarter's zpart inherits zt's bank (freed by the reciprocal
            # ~1.1us into this tail) instead of rd_ps's (freed by the rd cast
            # ~2.3us in), so the new quarter's first Z matmuls never stall on
            # this quarter's tail chain.
            rd_ps = zp.tile([1, 512], f32, tag="z", name="rd_ps")
            zt = zp.tile([128, 4], f32, tag="z", name="zt")
            rb_ps = zp.tile([128, 512], f32, tag="z", name="rb_ps")
            return (us, zsb, zt, rd_ps, rb_ps)

        def emit_tail_b(i0, pend, on_dve, last=False):
            # phase B (emitted after the NEXT block's U stream so none of
            # these PE matmuls can head-of-line block the score pipeline):
            # Z reduce+transpose, 1/Z, broadcast, y = us*(gamma/Z) + xr.
            us, zsb, zt, rd_ps, rb_ps = pend
            for c in range(4):
                nc.tensor.matmul(
                    zt[:, c:c + 1],
                    lhsT=zsb[:, 128 * c:128 * (c + 1)],
                    rhs=indq_sb,
                    start=True,
                    stop=True,
                )
            rt = rpool.tile([128, 4], bf16, tag="rt", name="rt")
            with nc.allow_low_precision(reason="1/Z in bf16: 0.4% on a 2e-2 budget"):
                nc.vector.reciprocal(rt[:], zt[:, 0:4])
            rd = rpool.tile([1, 512], bf16, tag="rd", name="rd")
            for c in range(4):
                nc.tensor.matmul(
                    rd_ps[0:1, 128 * c:128 * (c + 1)],
                    lhsT=rt[:, c:c + 1],
                    rhs=identg_sb,
                    start=True,
                    stop=True,
                )
            nc.vector.tensor_copy(rd[:], rd_ps[0:1, :])
            # single K=1 N=512 broadcast matmul (the 4-chunk split buys
            # nothing once rd exists as one row)
            nc.tensor.matmul(
                rb_ps[:, :],
                lhsT=onesr_sb,
                rhs=rd[0:1, :],
                start=True,
                stop=True,
            )
            if not on_dve:
                rb_sb = rpool.tile([128, 512], f32, tag="rb", name="rb_sb")
                nc.vector.tensor_copy(rb_sb[:], rb_ps[:, :])
            eng = nc.vector if on_dve else nc.gpsimd
            for ct in range(2):
                yt = ypool.tile([128, 512], f32, tag="y", name="yt")
                eng.tensor_mul(yt[:], us[ct][:], rb_ps[:, :] if on_dve else rb_sb[:])
                eng.tensor_add(yt[:], yt[:], xr_sb[ct][:, i0:i0 + 512])
                nc.sync.dma_start(
                    out=y_ap[ct * 128:(ct + 1) * 128, i0:i0 + 512], in_=yt[:]
                )

        # Software pipeline over 16 (quarter, J) blocks: the scores + exp for
        # block k+1 are emitted at high priority BEFORE block k's U/Z matmuls,
        # so the next block's scores can never be head-of-line blocked behind
        # this block's U stream or the quarter tail on the PE queue.
        u_t = None
        zpart = None
        pending = None
        e_cur = hoisted_e[0]
        for k in range(16):
            qd, J = divmod(k, 4)
            i0 = qd * 512
            if J == 0:
                u_t = [
                    up.tile([128, 512], f32, tag=f"u{ct}", name=f"u{ct}", bufs=1)
                    for ct in range(2)
                ]
                # Z partials: row band 32g accumulates sum over J of E[4J+g];
                # written by column-tiled concurrent ones-matmuls.
                zpart = zp.tile([128, 512], f32, tag="z", name="zpart")
            if k + 1 < len(hoisted_e):
                e_next = hoisted_e[k + 1]
            else:
                e_next = emit_s_exp(k + 1) if k < 15 else None

            def emit_u(gs):
                for g in gs:
                    jb = 4 * J + g
                    eh = e_cur[g // 2][:, (g % 2) * 512:(g % 2 + 1) * 512]
                    for ct in range(2):
                        nc.tensor.matmul(
                            u_t[ct][:, :],
                            lhsT=vT_sb[:, jb * C + ct * 128:jb * C + ct * 128 + 128],
                            rhs=eh,
                            start=(jb == 0),
                            stop=(jb == 15),
                        )

            def emit_z(gs):
                for g in gs:
                    eh = e_cur[g // 2][:, (g % 2) * 512:(g % 2 + 1) * 512]
                    nc.tensor.matmul(
                        zpart[32 * g:32 * (g + 1), :],
                        lhsT=ones_sb,
                        rhs=eh,
                        start=(J == 0),
                        stop=(J == 3),
                        tile_position=(0, 32 * g),
                        # 4 col-groups accumulate into disjoint 32-row bands of
                        # one bank; per-element has_written makes this safe on
                        # HW but the sim's group map is partition-blind.
                        skip_group_check=True,
                    )

            if k < 15:
                emit_u(range(4))
                # the 4 Z matmuls back-to-back so they pack into one
                # concurrent column-tiled group (~1 matmul of PE time for 4)
                emit_z(range(4))
            else:
                # last block: Z ahead of the second U half-stream (same exp
                # gate) so zpart completes ~0.9us before the U stream ends
                # and the final tail chain overlaps the last U matmuls.
                emit_u((0, 1))
                emit_z((0, 1))
                emit_z((2, 3))
                emit_u((2, 3))
            e_cur = e_next
            if pending is not None:
                # previous quarter's tail (pending is set at J==3, consumed at
                # the next block = J==0 of quarter qd), now safely behind this
                # block's score+U matmuls in queue order
                emit_tail_b((qd - 1) * 512, pending, on_dve=(qd - 1 >= 2))
                pending = None
            if J == 3:
                pending = emit_tail_a(u_t, zpart, last=(k == 15))
                if k == 15:
                    emit_tail_b(i0, pending, on_dve=True, last=True)
                    pending = None

    nc.compile()
    return nc


def get_nc():
    if "nc" not in _cache:
        _cache["nc"] = _build_nc()
    return _cache["nc"]


def make_in_maps(x, Wq, bq, Wk, bk, Wv, bv, gamma):
    import ml_dtypes

    bf = ml_dtypes.bfloat16
    x = np.asarray(x, dtype=np.float32)
    g = float(np.asarray(gamma, np.float32).reshape(-1)[0])
    cpack = np.zeros((128, 257), np.float32)
    cpack[0::32, 0] = 1.0                       # indicator
    cpack[:, 1:129] = g * np.eye(128)           # gamma * I
    cpack[:, 129:257] = 1.0                     # ones
    # bv folds into the residual: out = gamma*(Wv x E^T)/Z + (x + gamma*bv)
    xr_bias = (g * np.asarray(bv, np.float32)).reshape(1, C, 1)
    shared = {
        "wpack": np.ascontiguousarray(
            np.concatenate([np.tile(np.asarray(Wq, np.float32).T, (1, 4)),
                            np.tile(np.asarray(Wk, np.float32).T, (1, 4)),
                            np.asarray(Wv, np.float32).T],
                           axis=1)).astype(bf),
        "bqk": np.stack([np.tile(np.asarray(bq, np.float32), 4),
                         np.tile(np.asarray(bk, np.float32), 4)], axis=1),
        "cpack": cpack.astype(bf),
    }
    xrs = (x + xr_bias).astype(np.float32)
    return [
        dict(shared, xr=np.ascontiguousarray(xrs[b]),
             xb=np.ascontiguousarray(x[b]).astype(bf))
        for b in range(B)
    ]


def kernel(x, Wq, bq, Wk, bk, Wv, bv, gamma):
    from concourse.bass_utils import run_bass_kernel_spmd

    nc = get_nc()
    in_maps = make_in_maps(x, Wq, bq, Wk, bk, Wv, bv, gamma)
    res = run_bass_kernel_spmd(nc, in_maps, list(range(NCORES)))
    return np.stack([res.results[b]["y"] for b in range(B)], axis=0)
